# revision 1
# baseline (speedup 1.0000x reference)
"""Trainium2 Bass kernel for nn_Classical_autoencoder (patch MLP autoencoder + cosine fold).

Contract: kernel(**inputs) takes FULL inputs (img (32,1,512,512), W1 (16,4), b1 (4,),
W2 (4,4), b2 (4,), W3 (4,16), b3 (16,)) and returns the FULL (32,512,512) f32 output.
Internally: pure data-parallel over 8 NeuronCores, 4 images per core.

Math (per image):
  patches x = im2col(img, 4x4, stride 2)           # (255*255, 16)
  y = relu(relu(relu(x@W1+b1)@W2+b2)@W3+b3)        # (P, 16)
  S[i,j] = x.y / (max(|x|,eps)*max(|y|,eps))       # (255,255)
  out[r,c] = mean of S[i,j] for i in {r//2-1, r//2} & [0,255), j likewise
  (the overlapping fold with k=4,s=2 reduces exactly to this 2-tap box filter
   on S, upsampled 2x with 2x2-constant blocks)

V3 layout / engine plan (vs the v1 block-diag kernel):
  row tile RT [128=(32k+g), 8=(li), 2=(t), 256=(j)] bf16: partition 32k+g holds
      img row 16g+k+2li, col c split as c=2j+t.  j packed => DVE 2x mode on
      products; matmul views stay expressible as APs.
  |x|^2 via shared box filter (sq -> col-pair sums c2 -> 4-col sums s1) on DVE,
      then ONE cw matmul per ci (vs 4).
  ct contractions (dot, |y|^2, |x|^2) write a single PSUM bank each, 4 ci at
      partition offsets 32ci (tile_position) => tail ops are [128,510] not 4x[32,510].
  reciprocal -> reciprocal_approx_fast (HW reciprocal is ~8 cycles/elem).
  ReLUs split across ACT and GpSimd (GpSimd was idle); products/squares on DVE.
  Fold: DRAM bounce to row-pair layout, partition-offset adds, 2x2 upsample via
      stride-0 DMA reads, bf16 output (host upcasts).
"""

import sys

for _p in ("/opt/trn_rl_repo", "/root/.axon_site/_ro/trn_rl_repo"):
    if _p not in sys.path:
        sys.path.append(_p)

from contextlib import ExitStack

import numpy as np

import concourse.bass as bass
import concourse.tile as tile
from concourse import bacc, mybir

F32 = mybir.dt.float32
BF16 = mybir.dt.bfloat16
ADD = mybir.AluOpType.add
MULT = mybir.AluOpType.mult
MAX = mybir.AluOpType.max

IMG = 512
OH = 255
NSAMP = 4
NCORES = 8

# GPSIMD cannot read PSUM (BIR verifier) -> all relus on ACT/DVE.
# ysq is SBUF-only, so GpSimd can take a share of it.
YSQ_GPS = (True, True, False, False)  # per ci


def build_nc() -> bass.Bass:
    nc = bacc.Bacc()

    img4b = nc.declare_dram_parameter("img4b", [NSAMP, 128, 8, 2, 256], BF16, isOutput=False)[:]
    img4c = nc.declare_dram_parameter("img4c", [NSAMP, 128, 8, 2, 256], BF16, isOutput=False)[:]
    l1w = nc.declare_dram_parameter("l1w", [128, 4, 128], BF16, isOutput=False)[:]
    l2w = nc.declare_dram_parameter("l2w", [128, 128], BF16, isOutput=False)[:]
    l3w = nc.declare_dram_parameter("l3w", [128, 4, 128], BF16, isOutput=False)[:]
    b3v = nc.declare_dram_parameter("b3v", [128, 4], F32, isOutput=False)[:]
    cw = nc.declare_dram_parameter("cw", [128, 32], BF16, isOutput=False)[:]
    b1v = nc.declare_dram_parameter("b1v", [128, 1], F32, isOutput=False)[:]
    b2v = nc.declare_dram_parameter("b2v", [128, 1], F32, isOutput=False)[:]
    out4 = nc.declare_dram_parameter("out4", [NSAMP, IMG, IMG], BF16, isOutput=True)[:]

    with ExitStack() as ctx:
        tc = ctx.enter_context(tile.TileContext(nc))
        consts = ctx.enter_context(tc.tile_pool(name="consts", bufs=1))
        rows = ctx.enter_context(tc.tile_pool(name="rows", bufs=2))
        xsq = ctx.enter_context(tc.tile_pool(name="xsq", bufs=2))
        mlp = ctx.enter_context(tc.tile_pool(name="mlp", bufs=2))
        tailp = ctx.enter_context(tc.tile_pool(name="tailp", bufs=2))
        foldp = ctx.enter_context(tc.tile_pool(name="foldp", bufs=2))
        psz = ctx.enter_context(tc.tile_pool(name="psz", bufs=2, space="PSUM"))
        psz3 = ctx.enter_context(tc.tile_pool(name="psz3", bufs=2, space="PSUM"))
        psct = ctx.enter_context(tc.tile_pool(name="psct", bufs=1, space="PSUM"))
        dram = ctx.enter_context(tc.tile_pool(name="dram", bufs=2, space="DRAM"))

        # ---- constants ----
        l1w_t = consts.tile([128, 4, 128], BF16)
        nc.sync.dma_start(out=l1w_t, in_=l1w[:, :, :])
        l2w_t = consts.tile([128, 128], BF16)
        nc.sync.dma_start(out=l2w_t, in_=l2w[:, :])
        l3w_t = consts.tile([128, 4, 128], BF16)
        nc.sync.dma_start(out=l3w_t, in_=l3w[:, :, :])
        b3_t = consts.tile([128, 4], F32)
        nc.sync.dma_start(out=b3_t, in_=b3v[:, :])
        cw_t = consts.tile([128, 32], BF16)
        nc.sync.dma_start(out=cw_t, in_=cw[:, :])
        b1_t = consts.tile([128, 1], F32)
        nc.sync.dma_start(out=b1_t, in_=b1v[:, :])
        b2_t = consts.tile([128, 1], F32)
        nc.sync.dma_start(out=b2_t, in_=b2v[:, :])
        eps_t = consts.tile([128, 1], F32)
        nc.vector.memset(eps_t, 1e-12)

        def relu(out, z, bias, dve=False):
            if dve:
                nc.vector.tensor_scalar(out, z, bias, 0.0, ADD, MAX)
            else:
                nc.scalar.activation(out, z, mybir.ActivationFunctionType.Relu, bias=bias)

        # prefetched row tiles: rtb (cols 2j+t) and rtb2 (cols 2j+2+t, aligned
        # views for the l=2,3 product operands)
        rt_tiles = {}

        def fetch(s):
            # split li 0-1 first so ci=0 compute can start before the full tile lands
            rtb = rows.tile([128, 8, 2, 256], BF16, tag="rtb", name=f"rtb{s}")
            nc.sync.dma_start(out=rtb[:, 0:2], in_=img4b[s, :, 0:2, :, :])
            rtb2 = rows.tile([128, 8, 2, 256], BF16, tag="rtb2", name=f"rtb2_{s}")
            nc.sync.dma_start(out=rtb2[:, 0:2], in_=img4c[s, :, 0:2, :, :])
            nc.sync.dma_start(out=rtb[:, 2:8], in_=img4b[s, :, 2:8, :, :])
            nc.sync.dma_start(out=rtb2[:, 2:8], in_=img4c[s, :, 2:8, :, :])
            rt_tiles[s] = (rtb, rtb2)

        fetch(0)
        for s in range(NSAMP):
            rtb, rtb2 = rt_tiles.pop(s)
            rtb_flat = rtb.rearrange("p a t j -> p (a t j)")

            # ---- |x|^2 box filter prep (all li at once) ----
            sq = xsq.tile([128, 8, 2, 256], BF16, tag="sq")
            nc.vector.tensor_tensor(
                sq.rearrange("p a t j -> p (a t j)"), rtb_flat, rtb_flat, MULT
            )
            c2 = xsq.tile([128, 8, 256], BF16, tag="c2")
            nc.vector.tensor_tensor(c2, sq[:, :, 0, :], sq[:, :, 1, :], ADD)
            s1 = xsq.tile([128, 8, 255], BF16, tag="s1")
            nc.vector.tensor_tensor(s1, c2[:, :, 0:255], c2[:, :, 1:256], ADD)

            # ---- ct accumulator banks (one full 512-f32 bank each; 4 ci strips) ----
            ctd_b = psct.tile([128, 512], F32, tag="ctd")
            ctx_b = psct.tile([128, 512], F32, tag="ctx")
            cty_b = psct.tile([128, 512], F32, tag="cty")

            def xv(ci, l):
                # [128, 2(li), 255(j)] view: patch cols j at image col 2j+l
                return rtb[:, 2 * ci : 2 * ci + 2, l % 2, (l // 2) : (l // 2) + 255]

            def ct_d(ci, proda):
                tp = (0, 32 * ci)
                st = slice(32 * ci, 32 * ci + 32)
                for l in range(4):
                    nc.tensor.matmul(
                        ctd_b[st, 0:510], cw_t, proda[:, l % 2, l // 2, :, :],
                        start=(l == 0), stop=(l == 3), tile_position=tp,
                    )

            def ct_yx(ci, ysqa):
                tp = (0, 32 * ci)
                st = slice(32 * ci, 32 * ci + 32)
                for l in range(4):
                    nc.tensor.matmul(
                        cty_b[st, 0:510], cw_t, ysqa[:, l % 2, l // 2, :, :],
                        start=(l == 0), stop=(l == 3), tile_position=tp,
                    )
                nc.tensor.matmul(
                    ctx_b[st, 0:510], cw_t, s1[:, 2 * ci : 2 * ci + 2, :],
                    start=True, stop=True, tile_position=tp,
                )

            # Software pipeline: ci's ct matmuls are emitted one ci later,
            # slotted into the h1/h2 dependency windows so the PE never idles.
            pend = None
            for ci in range(4):
                # ---- layer 1 ----
                z1 = psz.tile([128, 510], F32, tag="z", name=f"z1_{s}_{ci}")
                for l in range(4):
                    nc.tensor.matmul(
                        z1, l1w_t[:, l, :], xv(ci, l), start=(l == 0), stop=(l == 3)
                    )
                if pend is not None:
                    ct_d(pend[0], pend[1])
                h1 = mlp.tile([128, 510], BF16, tag="h1", name=f"h1_{s}_{ci}")
                relu(h1, z1, b1_t[:, :])
                # ---- layer 2 ----
                z2 = psz.tile([128, 510], F32, tag="z", name=f"z2_{s}_{ci}")
                nc.tensor.matmul(z2, l2w_t[:, :], h1, start=True, stop=True)
                if pend is not None:
                    ct_yx(pend[0], pend[2])
                h2 = mlp.tile([128, 510], BF16, tag="h2", name=f"h2_{s}_{ci}")
                relu(h2, z2, b2_t[:, :])
                # ---- layer 3 + products ----
                yva = mlp.tile([128, 2, 2, 2, 255], BF16, tag="yva", name=f"yva_{s}_{ci}")
                for l in range(4):
                    z3 = psz3.tile([128, 510], F32, tag="z3", name=f"z3_{s}_{ci}_{l}")
                    nc.tensor.matmul(z3, l3w_t[:, l, :], h2, start=True, stop=True)
                    relu(
                        yva[:, l % 2, l // 2, :, :].rearrange("p a j -> p (a j)"),
                        z3, b3_t[:, l : l + 1],
                    )
                proda = mlp.tile([128, 2, 2, 2, 255], BF16, tag="proda", name=f"proda_{s}_{ci}")
                ysqa = mlp.tile([128, 2, 2, 2, 255], BF16, tag="ysqa", name=f"ysqa_{s}_{ci}")
                for l in range(4):
                    t, jo = l % 2, l // 2
                    src = rtb if jo == 0 else rtb2
                    nc.vector.tensor_tensor(
                        proda[:, t, jo, :, :],
                        src[:, 2 * ci : 2 * ci + 2, t, 0:255],
                        yva[:, t, jo, :, :], MULT,
                    )
                yva_f = yva.rearrange("p t a b j -> p (t a b j)")
                ysqa_f = ysqa.rearrange("p t a b j -> p (t a b j)")
                if YSQ_GPS[ci]:
                    nc.gpsimd.tensor_tensor(ysqa_f, yva_f, yva_f, MULT)
                else:
                    nc.vector.tensor_tensor(ysqa_f, yva_f, yva_f, MULT)
                pend = (ci, proda, ysqa)
            ct_d(pend[0], pend[1])
            ct_yx(pend[0], pend[2])

            # prefetch next image's row tiles before the tail/fold DMAs queue up
            if s + 1 < NSAMP:
                fetch(s + 1)

            # ---- cosine tail (batched over all 4 ci) ----
            # sim = ctd / (4|x| * |y|); the /4 pre-scales the fold average.
            sx = tailp.tile([128, 510], F32, tag="sx")
            nc.scalar.activation(
                sx, ctx_b[:, 0:510],
                mybir.ActivationFunctionType.Sqrt, bias=eps_t[:, :], scale=16.0,
            )
            sy = tailp.tile([128, 510], F32, tag="sy")
            nc.scalar.activation(
                sy, cty_b[:, 0:510],
                mybir.ActivationFunctionType.Sqrt, bias=eps_t[:, :],
            )
            m_ = tailp.tile([128, 510], F32, tag="m_")
            nc.gpsimd.tensor_tensor(m_, sx, sy, MULT)
            r_ = tailp.tile([128, 510], F32, tag="r_")
            nc.vector.reciprocal_approx_fast(r_, m_)
            simacc = tailp.tile([128, 2, 255], F32, tag="simacc")
            nc.vector.tensor_tensor(
                simacc.rearrange("p a j -> p (a j)"), ctd_b[:, 0:510], r_, MULT,
            )

            # ---- reorg S to row-pair layout via DRAM bounce ----
            # simacc[32ci+g, u, j] = S[8g+2ci+u, j]; bounce holds S row-major.
            sdram = dram.tile([256 * OH], F32, tag="sd")
            for ci in range(4):
                nc.sync.dma_start(
                    out=bass.AP(
                        tensor=sdram.tensor,
                        offset=sdram.offset + 2 * ci * OH,
                        ap=[[8 * OH, 32], [OH, 2], [1, OH]],
                    ),
                    in_=simacc[32 * ci : 32 * ci + 32, :, :],
                )
            simt = foldp.tile([128, 2, 256], F32, tag="simt")
            nc.sync.dma_start(
                out=simt[0:128, :, 0:255],
                in_=bass.AP(
                    tensor=sdram.tensor, offset=sdram.offset,
                    ap=[[2 * OH, 128], [OH, 2], [1, OH]],
                ),
            )

            # ---- fold cols: R[i,v] = S[i,v-1]+S[i,v], edges doubled ----
            rf = foldp.tile([128, 2, 256], F32, tag="rf")
            nc.vector.tensor_tensor(
                rf[:, :, 1:255], simt[:, :, 0:254], simt[:, :, 1:255], ADD
            )
            nc.scalar.activation(
                rf[:, :, 0:1], simt[:, :, 0:1],
                mybir.ActivationFunctionType.Copy, scale=2.0,
            )
            nc.scalar.activation(
                rf[:, :, 255:256], simt[:, :, 254:255],
                mybir.ActivationFunctionType.Copy, scale=2.0,
            )
            # S row 255 doesn't exist -> row 511 folds 2*R[254]
            nc.sync.dma_start(out=rf[127:128, 1, :], in_=rf[127:128, 0, :])

            # ---- fold rows: T[2p+u] = R[2p+u-1]+R[2p+u] ----
            # partition-shifted copy of odd rows: rfs[p] = R[2p-1] (rfs[0]=R[0])
            rfs = foldp.tile([128, 256], F32, tag="rfs")
            nc.sync.dma_start(out=rfs[1:128, :], in_=rf[0:127, 1, :])
            nc.sync.dma_start(out=rfs[0:1, :], in_=rf[0:1, 0, :])
            tf = foldp.tile([128, 2, 256], BF16, tag="tf")
            nc.vector.tensor_tensor(tf[:, 1, :], rf[:, 0, :], rf[:, 1, :], ADD)
            nc.vector.tensor_tensor(tf[:, 0, :], rfs, rf[:, 0, :], ADD)

            # ---- 2x2 upsample: duplicate cols on-chip, then 2 contiguous DMAs ----
            tf2 = foldp.tile([128, 2, 512], BF16, tag="tf2")
            tf2r = tf2.rearrange("p u (v cv) -> p u cv v", cv=2)
            nc.scalar.activation(
                tf2r[:, :, 0, :], tf, mybir.ActivationFunctionType.Copy
            )
            nc.vector.tensor_copy(tf2r[:, :, 1, :], tf)
            for ru in range(2):
                nc.sync.dma_start(
                    out=bass.AP(
                        tensor=out4.tensor,
                        offset=out4.offset + s * IMG * IMG + ru * IMG,
                        ap=[[4 * IMG, 128], [2 * IMG, 2], [1, IMG]],
                    ),
                    in_=bass.AP(
                        tensor=tf2.tensor,
                        offset=tf2.offset,
                        ap=[[1024, 128], [512, 2], [1, 512]],
                    ),
                )

    nc.finalize()
    return nc


def make_weight_inputs(W1, b1, W2, b2, W3, b3):
    """Host-side block-diagonal weight construction."""
    W1 = np.asarray(W1, np.float32)
    W2 = np.asarray(W2, np.float32)
    W3 = np.asarray(W3, np.float32)
    b1 = np.asarray(b1, np.float32)
    b2 = np.asarray(b2, np.float32)
    b3 = np.asarray(b3, np.float32)
    # partition orders: image/z3 rows p = 32k+g ; h1/h2 rows q = 32c+g
    l1w = np.zeros((128, 4, 128), np.float32)
    l2w = np.zeros((128, 128), np.float32)
    l3w = np.zeros((128, 4, 128), np.float32)
    b3v = np.zeros((128, 4), np.float32)
    cwm = np.zeros((128, 32), np.float32)
    for g in range(32):
        for l in range(4):
            for k in range(4):
                for c in range(4):
                    l1w[32 * k + g, l, 32 * c + g] = W1[4 * k + l, c]
                    l3w[32 * c + g, l, 32 * k + g] = W3[c, 4 * k + l]
                b3v[32 * k + g, l] = b3[4 * k + l]
                cwm[32 * k + g, g] = 1.0
        for c in range(4):
            for c2 in range(4):
                l2w[32 * c + g, 32 * c2 + g] = W2[c, c2]
    b1v = np.repeat(b1, 32).reshape(128, 1).astype(np.float32)
    b2v = np.repeat(b2, 32).reshape(128, 1).astype(np.float32)
    import ml_dtypes

    bf = ml_dtypes.bfloat16
    return {
        "l1w": l1w.astype(bf), "l2w": l2w.astype(bf), "l3w": l3w.astype(bf),
        "b3v": b3v, "cw": cwm.astype(bf), "b1v": b1v, "b2v": b2v,
    }


_NC = None


def get_nc():
    global _NC
    if _NC is None:
        _NC = build_nc()
    return _NC


def _bf16():
    import ml_dtypes

    return ml_dtypes.bfloat16


def gather_rows(img_n, shift=0):
    """(n,512,512) f32 -> (n,128,8,2,256) bf16: [p=32k+g, li, t, j] = img[16g+k+2li, 2j+shift+t]."""
    n = img_n.shape[0]
    pad = np.zeros((n, IMG + 4, IMG + 2), np.float32)
    pad[:, :IMG, :IMG] = img_n
    p = np.arange(128)
    li = np.arange(8)
    rows_idx = 16 * (p[:, None] % 32) + (p[:, None] // 32) + 2 * li[None, :]
    out = pad[:, rows_idx, shift : shift + IMG]     # (n,128,8,512)
    out = out.reshape(n, 128, 8, 256, 2).transpose(0, 1, 2, 4, 3)  # (n,128,8,2,256)
    return np.ascontiguousarray(out).astype(_bf16())


def kernel(img, W1, b1, W2, b2, W3, b3):
    from concourse.bass_utils import run_bass_kernel_spmd

    img = np.asarray(img, np.float32).reshape(32, IMG, IMG)
    wts = make_weight_inputs(W1, b1, W2, b2, W3, b3)
    nc = get_nc()
    core_ids = list(range(NCORES))
    in_maps = []
    for c in range(NCORES):
        chunk = img[c * NSAMP : (c + 1) * NSAMP]
        m = {"img4b": gather_rows(chunk), "img4c": gather_rows(chunk, shift=2)}
        m.update(wts)
        in_maps.append(m)
    res = run_bass_kernel_spmd(nc, in_maps, core_ids)
    out = np.concatenate([np.asarray(res.results[i]["out4"]) for i in range(NCORES)], axis=0)
    return out.astype(np.float32)



# revision 5
# speedup vs baseline: 1.0332x; 1.0332x over previous
"""Trainium2 Bass kernel for nn_Classical_autoencoder (patch MLP autoencoder + cosine fold).

Contract: kernel(**inputs) takes FULL inputs (img (32,1,512,512), W1 (16,4), b1 (4,),
W2 (4,4), b2 (4,), W3 (4,16), b3 (16,)) and returns the FULL (32,512,512) f32 output.
Internally: pure data-parallel over 8 NeuronCores, 4 images per core.

Math (per image):
  patches x = im2col(img, 4x4, stride 2)           # (255*255, 16)
  y = relu(relu(relu(x@W1+b1)@W2+b2)@W3+b3)        # (P, 16)
  S[i,j] = x.y / (max(|x|,eps)*max(|y|,eps))       # (255,255)
  out[r,c] = mean of S[i,j] for i in {r//2-1, r//2} & [0,255), j likewise
  (the overlapping fold with k=4,s=2 reduces exactly to this 2-tap box filter
   on S, upsampled 2x with 2x2-constant blocks)

V4 layout / engine plan (vs v3):
  img4c dropped: cols 2j+2+t are just rtb[..., j+1] (shift-by-one view), so
      one 512KB row tile per image instead of two.
  products: one 4-free-dim DVE op per ci (overlapping jo/j strides on the
      rtb read) instead of 4 ops -> 12 fewer DVE dispatches per image.
  cw4[32k+g, ci, 4g+ci] gather: ct matmul outputs land at partition q=4g+ci
      = patch rows (2q, 2q+1), i.e. already in fold row-pair order. The
      whole DRAM-bounce reorg of v3 (5 DMAs/image) disappears; all 4 ci
      accumulate into one full-width PSUM bank per quantity.
  single fused output DMA per image (row duplication via 0-stride read).
"""

import sys

for _p in ("/opt/trn_rl_repo", "/root/.axon_site/_ro/trn_rl_repo"):
    if _p not in sys.path:
        sys.path.append(_p)

from contextlib import ExitStack

import numpy as np

import concourse.bass as bass
import concourse.tile as tile
from concourse import bacc, mybir

F32 = mybir.dt.float32
BF16 = mybir.dt.bfloat16
ADD = mybir.AluOpType.add
MULT = mybir.AluOpType.mult
MAX = mybir.AluOpType.max

IMG = 512
OH = 255
NSAMP = 4
NCORES = 8

# GPSIMD cannot read PSUM (BIR verifier) -> all relus on ACT/DVE.
# ysq is SBUF-only, so GpSimd can take a share of it.
YSQ_GPS = (True, True, True, False)  # per ci


def build_nc() -> bass.Bass:
    nc = bacc.Bacc()

    img4b = nc.declare_dram_parameter("img4b", [NSAMP, 128, 8, 2, 256], BF16, isOutput=False)[:]
    l1w = nc.declare_dram_parameter("l1w", [128, 4, 128], BF16, isOutput=False)[:]
    l2w = nc.declare_dram_parameter("l2w", [128, 128], BF16, isOutput=False)[:]
    l3w = nc.declare_dram_parameter("l3w", [128, 4, 128], BF16, isOutput=False)[:]
    b3v = nc.declare_dram_parameter("b3v", [128, 4], F32, isOutput=False)[:]
    cw4 = nc.declare_dram_parameter("cw4", [128, 4, 128], BF16, isOutput=False)[:]
    b1v = nc.declare_dram_parameter("b1v", [128, 1], F32, isOutput=False)[:]
    b2v = nc.declare_dram_parameter("b2v", [128, 1], F32, isOutput=False)[:]
    out4 = nc.declare_dram_parameter("out4", [NSAMP, IMG, IMG], BF16, isOutput=True)[:]

    with ExitStack() as ctx:
        tc = ctx.enter_context(tile.TileContext(nc))
        consts = ctx.enter_context(tc.tile_pool(name="consts", bufs=1))
        rows = ctx.enter_context(tc.tile_pool(name="rows", bufs=2))
        xsq = ctx.enter_context(tc.tile_pool(name="xsq", bufs=2))
        mlp = ctx.enter_context(tc.tile_pool(name="mlp", bufs=2))
        tailp = ctx.enter_context(tc.tile_pool(name="tailp", bufs=2))
        foldp = ctx.enter_context(tc.tile_pool(name="foldp", bufs=2))
        psz = ctx.enter_context(tc.tile_pool(name="psz", bufs=2, space="PSUM"))
        psz3 = ctx.enter_context(tc.tile_pool(name="psz3", bufs=3, space="PSUM"))
        psct = ctx.enter_context(tc.tile_pool(name="psct", bufs=1, space="PSUM"))

        # ---- constants ----
        l1w_t = consts.tile([128, 4, 128], BF16)
        nc.sync.dma_start(out=l1w_t, in_=l1w[:, :, :])
        l2w_t = consts.tile([128, 128], BF16)
        nc.sync.dma_start(out=l2w_t, in_=l2w[:, :])
        l3w_t = consts.tile([128, 4, 128], BF16)
        nc.sync.dma_start(out=l3w_t, in_=l3w[:, :, :])
        b3_t = consts.tile([128, 4], F32)
        nc.sync.dma_start(out=b3_t, in_=b3v[:, :])
        cw_t = consts.tile([128, 4, 128], BF16)
        nc.sync.dma_start(out=cw_t, in_=cw4[:, :, :])
        b1_t = consts.tile([128, 1], F32)
        nc.sync.dma_start(out=b1_t, in_=b1v[:, :])
        b2_t = consts.tile([128, 1], F32)
        nc.sync.dma_start(out=b2_t, in_=b2v[:, :])
        eps_t = consts.tile([128, 1], F32)
        nc.vector.memset(eps_t, 1e-12)

        def relu(out, z, bias):
            nc.scalar.activation(out, z, mybir.ActivationFunctionType.Relu, bias=bias)

        rt_tiles = {}

        def fetch(s):
            # split li 0-1 first so ci=0 compute can start before the full tile lands
            rtb = rows.tile([128, 8, 2, 256], BF16, tag="rtb", name=f"rtb{s}")
            nc.sync.dma_start(out=rtb[:, 0:2], in_=img4b[s, :, 0:2, :, :])
            nc.sync.dma_start(out=rtb[:, 2:8], in_=img4b[s, :, 2:8, :, :])
            rt_tiles[s] = rtb

        fetch(0)
        for s in range(NSAMP):
            rtb = rt_tiles.pop(s)
            rtb_flat = rtb.rearrange("p a t j -> p (a t j)")

            # ---- |x|^2 box filter prep (all li at once) ----
            sq = xsq.tile([128, 8, 2, 256], BF16, tag="sq")
            nc.vector.tensor_tensor(
                sq.rearrange("p a t j -> p (a t j)"), rtb_flat, rtb_flat, MULT
            )
            c2 = xsq.tile([128, 8, 256], BF16, tag="c2")
            nc.vector.tensor_tensor(c2, sq[:, :, 0, :], sq[:, :, 1, :], ADD)
            s1 = xsq.tile([128, 8, 255], BF16, tag="s1")
            nc.vector.tensor_tensor(s1, c2[:, :, 0:255], c2[:, :, 1:256], ADD)

            # ---- ct accumulator banks: all ci accumulate into one full bank ----
            # out partition q = 4g+ci holds patch rows (2q, 2q+1) -> fold order.
            ctd_b = psct.tile([128, 512], F32, tag="ctd")
            ctx_b = psct.tile([128, 512], F32, tag="ctx")
            cty_b = psct.tile([128, 512], F32, tag="cty")

            def xv(ci, l):
                # [128, 2(li), 255(j)] view: patch cols j at image col 2j+l
                return rtb[:, 2 * ci : 2 * ci + 2, l % 2, (l // 2) : (l // 2) + 255]

            def ct_d(ci, proda):
                for l in range(4):
                    nc.tensor.matmul(
                        ctd_b[:, 0:510], cw_t[:, ci, :], proda[:, l % 2, l // 2, :, :],
                        start=(ci == 0 and l == 0), stop=(ci == 3 and l == 3),
                    )

            def ct_yx(ci, ysqa):
                for l in range(4):
                    nc.tensor.matmul(
                        cty_b[:, 0:510], cw_t[:, ci, :], ysqa[:, l % 2, l // 2, :, :],
                        start=(ci == 0 and l == 0), stop=(ci == 3 and l == 3),
                    )
                nc.tensor.matmul(
                    ctx_b[:, 0:510], cw_t[:, ci, :], s1[:, 2 * ci : 2 * ci + 2, :],
                    start=(ci == 0), stop=(ci == 3),
                )

            # Software pipeline: ci's ct matmuls are emitted one ci later,
            # slotted into the h1/h2 dependency windows so the PE never idles.
            pend = None
            for ci in range(4):
                # ---- layer 1 ----
                z1 = psz.tile([128, 510], F32, tag="z", name=f"z1_{s}_{ci}")
                for l in range(4):
                    nc.tensor.matmul(
                        z1, l1w_t[:, l, :], xv(ci, l), start=(l == 0), stop=(l == 3)
                    )
                if pend is not None:
                    ct_d(pend[0], pend[1])
                h1 = mlp.tile([128, 510], BF16, tag="h1", name=f"h1_{s}_{ci}")
                relu(h1, z1, b1_t[:, :])
                # ---- layer 2 ----
                z2 = psz.tile([128, 510], F32, tag="z", name=f"z2_{s}_{ci}")
                nc.tensor.matmul(z2, l2w_t[:, :], h1, start=True, stop=True)
                if pend is not None:
                    ct_yx(pend[0], pend[2])
                h2 = mlp.tile([128, 510], BF16, tag="h2", name=f"h2_{s}_{ci}")
                relu(h2, z2, b2_t[:, :])
                # ---- layer 3 + products ----
                yva = mlp.tile([128, 2, 2, 2, 255], BF16, tag="yva", name=f"yva_{s}_{ci}")
                for l in range(4):
                    z3 = psz3.tile([128, 510], F32, tag="z3", name=f"z3_{s}_{ci}_{l}")
                    nc.tensor.matmul(z3, l3w_t[:, l, :], h2, start=True, stop=True)
                    relu(
                        yva[:, l % 2, l // 2, :, :].rearrange("p a j -> p (a j)"),
                        z3, b3_t[:, l : l + 1],
                    )
                # products: one op per t; x view [jo, li, j] with overlapping jo/j strides
                proda = mlp.tile([128, 2, 2, 2, 255], BF16, tag="proda", name=f"proda_{s}_{ci}")
                for t in range(2):
                    xview = bass.AP(
                        tensor=rtb.tensor,
                        offset=rtb.offset + 2 * ci * 512 + t * 256,
                        ap=[[4096, 128], [1, 2], [512, 2], [1, 255]],
                    )
                    nc.vector.tensor_tensor(proda[:, t, :, :, :], xview, yva[:, t, :, :, :], MULT)
                yva_f = yva.rearrange("p t a b j -> p (t a b j)")
                ysqa = mlp.tile([128, 2, 2, 2, 255], BF16, tag="ysqa", name=f"ysqa_{s}_{ci}")
                ysqa_f = ysqa.rearrange("p t a b j -> p (t a b j)")
                if YSQ_GPS[ci]:
                    nc.gpsimd.tensor_tensor(ysqa_f, yva_f, yva_f, MULT)
                else:
                    nc.vector.tensor_tensor(ysqa_f, yva_f, yva_f, MULT)
                pend = (ci, proda, ysqa)
            ct_d(pend[0], pend[1])
            ct_yx(pend[0], pend[2])

            # prefetch next image's row tiles before the tail DMAs queue up
            if s + 1 < NSAMP:
                fetch(s + 1)

            # ---- cosine tail (partition q=4g+ci holds S rows 2q, 2q+1) ----
            # sim = ctd / (4|x| * |y|); the /4 pre-scales the fold average.
            sx = tailp.tile([128, 510], F32, tag="sx")
            nc.scalar.activation(
                sx, ctx_b[:, 0:510],
                mybir.ActivationFunctionType.Sqrt, bias=eps_t[:, :], scale=16.0,
            )
            sy = tailp.tile([128, 510], F32, tag="sy")
            nc.scalar.activation(
                sy, cty_b[:, 0:510],
                mybir.ActivationFunctionType.Sqrt, bias=eps_t[:, :],
            )
            m_ = tailp.tile([128, 510], F32, tag="m_")
            nc.gpsimd.tensor_tensor(m_, sx, sy, MULT)
            r_ = tailp.tile([128, 510], F32, tag="r_")
            nc.vector.reciprocal_approx_fast(r_, m_)
            simacc = tailp.tile([128, 2, 255], F32, tag="simacc")
            nc.vector.tensor_tensor(
                simacc.rearrange("p a j -> p (a j)"), ctd_b[:, 0:510], r_, MULT,
            )

            # ---- fold cols: R[q,u,v] = S[2q+u,v-1]+S[2q+u,v], edges doubled ----
            rf = foldp.tile([128, 2, 256], F32, tag="rf")
            nc.vector.tensor_tensor(
                rf[:, :, 1:255], simacc[:, :, 0:254], simacc[:, :, 1:255], ADD
            )
            nc.scalar.activation(
                rf[:, :, 0:1], simacc[:, :, 0:1],
                mybir.ActivationFunctionType.Copy, scale=2.0,
            )
            nc.scalar.activation(
                rf[:, :, 255:256], simacc[:, :, 254:255],
                mybir.ActivationFunctionType.Copy, scale=2.0,
            )
            # S row 255 doesn't exist -> row 511 folds 2*R[254]
            nc.sync.dma_start(out=rf[127:128, 1, :], in_=rf[127:128, 0, :])

            # ---- fold rows: T[2q+u] = R[2q+u-1]+R[2q+u] ----
            # partition-shifted copy of odd rows: rfs[q] = R[2q-1] (rfs[0]=R[0])
            rfs = foldp.tile([128, 256], F32, tag="rfs")
            nc.sync.dma_start(out=rfs[1:128, :], in_=rf[0:127, 1, :])
            nc.sync.dma_start(out=rfs[0:1, :], in_=rf[0:1, 0, :])
            tf = foldp.tile([128, 2, 256], BF16, tag="tf")
            nc.vector.tensor_tensor(tf[:, 1, :], rf[:, 0, :], rf[:, 1, :], ADD)
            nc.vector.tensor_tensor(tf[:, 0, :], rfs, rf[:, 0, :], ADD)

            # ---- 2x2 upsample: duplicate cols on-chip, one fused output DMA ----
            tf2 = foldp.tile([128, 2, 512], BF16, tag="tf2")
            tf2r = tf2.rearrange("p u (v cv) -> p u cv v", cv=2)
            nc.scalar.activation(
                tf2r[:, :, 0, :], tf, mybir.ActivationFunctionType.Copy
            )
            nc.vector.tensor_copy(tf2r[:, :, 1, :], tf)
            for ru in range(2):
                nc.sync.dma_start(
                    out=bass.AP(
                        tensor=out4.tensor,
                        offset=out4.offset + s * IMG * IMG + ru * IMG,
                        ap=[[4 * IMG, 128], [2 * IMG, 2], [1, IMG]],
                    ),
                    in_=bass.AP(
                        tensor=tf2.tensor,
                        offset=tf2.offset,
                        ap=[[1024, 128], [512, 2], [1, 512]],
                    ),
                )

    nc.finalize()
    return nc


def make_weight_inputs(W1, b1, W2, b2, W3, b3):
    """Host-side block-diagonal weight construction."""
    W1 = np.asarray(W1, np.float32)
    W2 = np.asarray(W2, np.float32)
    W3 = np.asarray(W3, np.float32)
    b1 = np.asarray(b1, np.float32)
    b2 = np.asarray(b2, np.float32)
    b3 = np.asarray(b3, np.float32)
    # partition orders: image/z3 rows p = 32k+g ; h1/h2 rows q = 32c+g
    l1w = np.zeros((128, 4, 128), np.float32)
    l2w = np.zeros((128, 128), np.float32)
    l3w = np.zeros((128, 4, 128), np.float32)
    b3v = np.zeros((128, 4), np.float32)
    cwm = np.zeros((128, 4, 128), np.float32)
    for g in range(32):
        for l in range(4):
            for k in range(4):
                for c in range(4):
                    l1w[32 * k + g, l, 32 * c + g] = W1[4 * k + l, c]
                    l3w[32 * c + g, l, 32 * k + g] = W3[c, 4 * k + l]
                b3v[32 * k + g, l] = b3[4 * k + l]
        for k in range(4):
            for ci in range(4):
                cwm[32 * k + g, ci, 4 * g + ci] = 1.0
        for c in range(4):
            for c2 in range(4):
                l2w[32 * c + g, 32 * c2 + g] = W2[c, c2]
    b1v = np.repeat(b1, 32).reshape(128, 1).astype(np.float32)
    b2v = np.repeat(b2, 32).reshape(128, 1).astype(np.float32)
    import ml_dtypes

    bf = ml_dtypes.bfloat16
    return {
        "l1w": l1w.astype(bf), "l2w": l2w.astype(bf), "l3w": l3w.astype(bf),
        "b3v": b3v, "cw4": cwm.astype(bf), "b1v": b1v, "b2v": b2v,
    }


_NC = None


def get_nc():
    global _NC
    if _NC is None:
        _NC = build_nc()
    return _NC


def _bf16():
    import ml_dtypes

    return ml_dtypes.bfloat16


def gather_rows(img_n):
    """(n,512,512) f32 -> (n,128,8,2,256) bf16: [p=32k+g, li, t, j] = img[16g+k+2li, 2j+t]."""
    n = img_n.shape[0]
    pad = np.zeros((n, IMG + 4, IMG), np.float32)
    pad[:, :IMG, :] = img_n
    p = np.arange(128)
    li = np.arange(8)
    rows_idx = 16 * (p[:, None] % 32) + (p[:, None] // 32) + 2 * li[None, :]
    out = pad[:, rows_idx, :]                       # (n,128,8,512)
    out = out.reshape(n, 128, 8, 256, 2).transpose(0, 1, 2, 4, 3)  # (n,128,8,2,256)
    return np.ascontiguousarray(out).astype(_bf16())


def make_in_maps(img, wts):
    in_maps = []
    for c in range(NCORES):
        chunk = img[c * NSAMP : (c + 1) * NSAMP]
        m = {"img4b": gather_rows(chunk)}
        m.update(wts)
        in_maps.append(m)
    return in_maps


def kernel(img, W1, b1, W2, b2, W3, b3):
    from concourse.bass_utils import run_bass_kernel_spmd

    img = np.asarray(img, np.float32).reshape(32, IMG, IMG)
    wts = make_weight_inputs(W1, b1, W2, b2, W3, b3)
    nc = get_nc()
    in_maps = make_in_maps(img, wts)
    res = run_bass_kernel_spmd(nc, in_maps, list(range(NCORES)))
    out = np.concatenate([np.asarray(res.results[i]["out4"]) for i in range(NCORES)], axis=0)
    return out.astype(np.float32)


# revision 7
# speedup vs baseline: 1.0511x; 1.0173x over previous
"""Trainium2 Bass kernel for nn_Classical_autoencoder (patch MLP autoencoder + cosine fold).

Contract: kernel(**inputs) takes FULL inputs (img (32,1,512,512), W1 (16,4), b1 (4,),
W2 (4,4), b2 (4,), W3 (4,16), b3 (16,)) and returns the FULL (32,512,512) f32 output.
Internally: pure data-parallel over 8 NeuronCores, 4 images per core.

Math (per image):
  patches x = im2col(img, 4x4, stride 2)           # (255*255, 16)
  y = relu(relu(relu(x@W1+b1)@W2+b2)@W3+b3)        # (P, 16)
  S[i,j] = x.y / (max(|x|,eps)*max(|y|,eps))       # (255,255)
  out[r,c] = mean of S[i,j] for i in {r//2-1, r//2} & [0,255), j likewise
  (the overlapping fold with k=4,s=2 reduces exactly to this 2-tap box filter
   on S, upsampled 2x with 2x2-constant blocks)

V4 layout / engine plan (vs v3):
  img4c dropped: cols 2j+2+t are just rtb[..., j+1] (shift-by-one view), so
      one 512KB row tile per image instead of two.
  products: one 4-free-dim DVE op per ci (overlapping jo/j strides on the
      rtb read) instead of 4 ops -> 12 fewer DVE dispatches per image.
  cw4[32k+g, ci, 4g+ci] gather: ct matmul outputs land at partition q=4g+ci
      = patch rows (2q, 2q+1), i.e. already in fold row-pair order. The
      whole DRAM-bounce reorg of v3 (5 DMAs/image) disappears; all 4 ci
      accumulate into one full-width PSUM bank per quantity.
  single fused output DMA per image (row duplication via 0-stride read).
"""

import sys

for _p in ("/opt/trn_rl_repo", "/root/.axon_site/_ro/trn_rl_repo"):
    if _p not in sys.path:
        sys.path.append(_p)

from contextlib import ExitStack

import numpy as np

import concourse.bass as bass
import concourse.tile as tile
from concourse import bacc, mybir

F32 = mybir.dt.float32
BF16 = mybir.dt.bfloat16
ADD = mybir.AluOpType.add
MULT = mybir.AluOpType.mult
MAX = mybir.AluOpType.max

IMG = 512
OH = 255
NSAMP = 4
NCORES = 8

# GPSIMD cannot read PSUM (BIR verifier) -> all relus on ACT/DVE.
# ysq is SBUF-only, so GpSimd can take a share of it.
YSQ_GPS = (True, True, True, False)  # per ci


def build_nc() -> bass.Bass:
    nc = bacc.Bacc()

    img4b = nc.declare_dram_parameter("img4b", [NSAMP, 128, 8, 2, 256], BF16, isOutput=False)[:]
    l1w = nc.declare_dram_parameter("l1w", [128, 4, 128], BF16, isOutput=False)[:]
    l2w = nc.declare_dram_parameter("l2w", [128, 128], BF16, isOutput=False)[:]
    l3w = nc.declare_dram_parameter("l3w", [128, 4, 128], BF16, isOutput=False)[:]
    b3v = nc.declare_dram_parameter("b3v", [128, 4], F32, isOutput=False)[:]
    cw4 = nc.declare_dram_parameter("cw4", [128, 4, 128], BF16, isOutput=False)[:]
    b1v = nc.declare_dram_parameter("b1v", [128, 1], F32, isOutput=False)[:]
    b2v = nc.declare_dram_parameter("b2v", [128, 1], F32, isOutput=False)[:]
    out4 = nc.declare_dram_parameter("out4", [NSAMP, IMG, IMG], BF16, isOutput=True)[:]

    with ExitStack() as ctx:
        tc = ctx.enter_context(tile.TileContext(nc))
        consts = ctx.enter_context(tc.tile_pool(name="consts", bufs=1))
        rows = ctx.enter_context(tc.tile_pool(name="rows", bufs=2))
        xsq = ctx.enter_context(tc.tile_pool(name="xsq", bufs=2))
        mlp = ctx.enter_context(tc.tile_pool(name="mlp", bufs=2))
        tailp = ctx.enter_context(tc.tile_pool(name="tailp", bufs=2))
        foldp = ctx.enter_context(tc.tile_pool(name="foldp", bufs=2))
        psz = ctx.enter_context(tc.tile_pool(name="psz", bufs=2, space="PSUM"))
        psz3 = ctx.enter_context(tc.tile_pool(name="psz3", bufs=3, space="PSUM"))
        psct = ctx.enter_context(tc.tile_pool(name="psct", bufs=1, space="PSUM"))

        rt_tiles = {}

        def fetch(s):
            # split li 0-1 first so ci=0 compute can start before the full tile lands
            rtb = rows.tile([128, 8, 2, 256], BF16, tag="rtb", name=f"rtb{s}")
            nc.sync.dma_start(out=rtb[:, 0:2], in_=img4b[s, :, 0:2, :, :])
            nc.sync.dma_start(out=rtb[:, 2:8], in_=img4b[s, :, 2:8, :, :])
            rt_tiles[s] = rtb

        # ---- constants (first fetch launched ahead; l1w/b1 next on the
        # sync queue; the rest spread over idle engine queues) ----
        fetch(0)
        l1w_t = consts.tile([128, 4, 128], BF16)
        nc.sync.dma_start(out=l1w_t, in_=l1w[:, :, :])
        b1_t = consts.tile([128, 1], F32)
        nc.sync.dma_start(out=b1_t, in_=b1v[:, :])
        l2w_t = consts.tile([128, 128], BF16)
        nc.gpsimd.dma_start(out=l2w_t, in_=l2w[:, :])
        b2_t = consts.tile([128, 1], F32)
        nc.gpsimd.dma_start(out=b2_t, in_=b2v[:, :])
        l3w_t = consts.tile([128, 4, 128], BF16)
        nc.scalar.dma_start(out=l3w_t, in_=l3w[:, :, :])
        b3_t = consts.tile([128, 4], F32)
        nc.scalar.dma_start(out=b3_t, in_=b3v[:, :])
        cw_t = consts.tile([128, 4, 128], BF16)
        nc.gpsimd.dma_start(out=cw_t, in_=cw4[:, :, :])
        eps_t = consts.tile([128, 1], F32)
        nc.vector.memset(eps_t, 1e-12)

        def relu(out, z, bias):
            nc.scalar.activation(out, z, mybir.ActivationFunctionType.Relu, bias=bias)

        # per-sample live state for the cross-image pipeline
        ct_banks = {}
        s1_tiles = {}
        sim_tiles = {}

        def ct_d(ps, ci, proda):
            ctd_b = ct_banks[ps][0]
            for l in range(4):
                nc.tensor.matmul(
                    ctd_b[:, 0:510], cw_t[:, ci, :], proda[:, l % 2, l // 2, :, :],
                    start=(ci == 0 and l == 0), stop=(ci == 3 and l == 3),
                )

        def ct_yx(ps, ci, ysqa):
            _, ctx_b, cty_b = ct_banks[ps]
            for l in range(4):
                nc.tensor.matmul(
                    cty_b[:, 0:510], cw_t[:, ci, :], ysqa[:, l % 2, l // 2, :, :],
                    start=(ci == 0 and l == 0), stop=(ci == 3 and l == 3),
                )
            nc.tensor.matmul(
                ctx_b[:, 0:510], cw_t[:, ci, :], s1_tiles[ps][:, 2 * ci : 2 * ci + 2, :],
                start=(ci == 0), stop=(ci == 3),
            )

        def alloc_ct(ps):
            # out partition q = 4g+ci holds patch rows (2q, 2q+1) -> fold order.
            ct_banks[ps] = (
                psct.tile([128, 512], F32, tag="ctd", name=f"ctd{ps}"),
                psct.tile([128, 512], F32, tag="ctx", name=f"ctx{ps}"),
                psct.tile([128, 512], F32, tag="cty", name=f"cty{ps}"),
            )

        def emit_sim(ps):
            # cosine tail (PSUM readers): sim = ctd / (4|x| * |y|).
            # The /4 pre-scales the fold average.
            ctd_b, ctx_b, cty_b = ct_banks.pop(ps)
            s1_tiles.pop(ps)
            sx = tailp.tile([128, 510], F32, tag="sx")
            nc.scalar.activation(
                sx, ctx_b[:, 0:510],
                mybir.ActivationFunctionType.Sqrt, bias=eps_t[:, :], scale=16.0,
            )
            sy = tailp.tile([128, 510], F32, tag="sy")
            nc.scalar.activation(
                sy, cty_b[:, 0:510],
                mybir.ActivationFunctionType.Sqrt, bias=eps_t[:, :],
            )
            m_ = tailp.tile([128, 510], F32, tag="m_")
            nc.gpsimd.tensor_tensor(m_, sx, sy, MULT)
            r_ = tailp.tile([128, 510], F32, tag="r_")
            nc.vector.reciprocal_approx_fast(r_, m_)
            simacc = tailp.tile([128, 2, 255], F32, tag="simacc")
            nc.vector.tensor_tensor(
                simacc.rearrange("p a j -> p (a j)"), ctd_b[:, 0:510], r_, MULT,
            )
            sim_tiles[ps] = simacc

        def emit_fold(ps):
            # fold cols: R[q,u,v] = S[2q+u,v-1]+S[2q+u,v], edges doubled
            simacc = sim_tiles.pop(ps)
            rf = foldp.tile([128, 2, 256], F32, tag="rf")
            nc.vector.tensor_tensor(
                rf[:, :, 1:255], simacc[:, :, 0:254], simacc[:, :, 1:255], ADD
            )
            nc.scalar.activation(
                rf[:, :, 0:1], simacc[:, :, 0:1],
                mybir.ActivationFunctionType.Copy, scale=2.0,
            )
            nc.scalar.activation(
                rf[:, :, 255:256], simacc[:, :, 254:255],
                mybir.ActivationFunctionType.Copy, scale=2.0,
            )
            # S row 255 doesn't exist -> row 511 folds 2*R[254]
            nc.sync.dma_start(out=rf[127:128, 1, :], in_=rf[127:128, 0, :])
            # fold rows: T[2q+u] = R[2q+u-1]+R[2q+u] via partition-shifted
            # copy of odd rows: rfs[q] = R[2q-1] (rfs[0]=R[0])
            rfs = foldp.tile([128, 256], F32, tag="rfs")
            nc.sync.dma_start(out=rfs[1:128, :], in_=rf[0:127, 1, :])
            nc.sync.dma_start(out=rfs[0:1, :], in_=rf[0:1, 0, :])
            tf = foldp.tile([128, 2, 256], BF16, tag="tf")
            nc.vector.tensor_tensor(tf[:, 1, :], rf[:, 0, :], rf[:, 1, :], ADD)
            nc.vector.tensor_tensor(tf[:, 0, :], rfs, rf[:, 0, :], ADD)
            # 2x2 upsample: duplicate cols on-chip, two row-pair output DMAs
            tf2 = foldp.tile([128, 2, 512], BF16, tag="tf2")
            tf2r = tf2.rearrange("p u (v cv) -> p u cv v", cv=2)
            nc.scalar.activation(
                tf2r[:, :, 0, :], tf, mybir.ActivationFunctionType.Copy
            )
            nc.vector.tensor_copy(tf2r[:, :, 1, :], tf)
            for ru in range(2):
                nc.sync.dma_start(
                    out=bass.AP(
                        tensor=out4.tensor,
                        offset=out4.offset + ps * IMG * IMG + ru * IMG,
                        ap=[[4 * IMG, 128], [2 * IMG, 2], [1, IMG]],
                    ),
                    in_=bass.AP(
                        tensor=tf2.tensor,
                        offset=tf2.offset,
                        ap=[[1024, 128], [512, 2], [1, 512]],
                    ),
                )

        # Software pipeline: ci's ct matmuls are emitted one ci later (crossing
        # image boundaries), slotted into the h1/h2 dependency windows so the
        # PE never idles; the previous image's tail rides along at ci 1-2.
        pend = None
        for s in range(NSAMP):
            rtb = rt_tiles.pop(s)
            rtb_flat = rtb.rearrange("p a t j -> p (a t j)")

            # ---- |x|^2 box filter prep (all li at once) ----
            sq = xsq.tile([128, 8, 2, 256], BF16, tag="sq")
            nc.vector.tensor_tensor(
                sq.rearrange("p a t j -> p (a t j)"), rtb_flat, rtb_flat, MULT
            )
            c2 = xsq.tile([128, 8, 256], BF16, tag="c2")
            nc.vector.tensor_tensor(c2, sq[:, :, 0, :], sq[:, :, 1, :], ADD)
            s1 = xsq.tile([128, 8, 255], BF16, tag="s1")
            nc.vector.tensor_tensor(s1, c2[:, :, 0:255], c2[:, :, 1:256], ADD)
            s1_tiles[s] = s1

            def xv(ci, l):
                # [128, 2(li), 255(j)] view: patch cols j at image col 2j+l
                return rtb[:, 2 * ci : 2 * ci + 2, l % 2, (l // 2) : (l // 2) + 255]

            for ci in range(4):
                if ci == 1:
                    # previous image's PSUM readers, then reclaim its banks
                    if s > 0:
                        emit_sim(s - 1)
                    alloc_ct(s)
                if ci == 2 and s > 0:
                    emit_fold(s - 1)
                # ---- layer 1 ----
                z1 = psz.tile([128, 510], F32, tag="z", name=f"z1_{s}_{ci}")
                for l in range(4):
                    nc.tensor.matmul(
                        z1, l1w_t[:, l, :], xv(ci, l), start=(l == 0), stop=(l == 3)
                    )
                if pend is not None:
                    ct_d(*pend[:3])
                h1 = mlp.tile([128, 510], BF16, tag="h1", name=f"h1_{s}_{ci}")
                relu(h1, z1, b1_t[:, :])
                # ---- layer 2 ----
                z2 = psz.tile([128, 510], F32, tag="z", name=f"z2_{s}_{ci}")
                nc.tensor.matmul(z2, l2w_t[:, :], h1, start=True, stop=True)
                if pend is not None:
                    ct_yx(pend[0], pend[1], pend[3])
                h2 = mlp.tile([128, 510], BF16, tag="h2", name=f"h2_{s}_{ci}")
                relu(h2, z2, b2_t[:, :])
                # ---- layer 3 + products ----
                yva = mlp.tile([128, 2, 2, 2, 255], BF16, tag="yva", name=f"yva_{s}_{ci}")
                for l in range(4):
                    z3 = psz3.tile([128, 510], F32, tag="z3", name=f"z3_{s}_{ci}_{l}")
                    nc.tensor.matmul(z3, l3w_t[:, l, :], h2, start=True, stop=True)
                    relu(
                        yva[:, l % 2, l // 2, :, :].rearrange("p a j -> p (a j)"),
                        z3, b3_t[:, l : l + 1],
                    )
                # products: one op per t; x view [jo, li, j] with overlapping jo/j strides
                proda = mlp.tile([128, 2, 2, 2, 255], BF16, tag="proda", name=f"proda_{s}_{ci}")
                for t in range(2):
                    xview = bass.AP(
                        tensor=rtb.tensor,
                        offset=rtb.offset + 2 * ci * 512 + t * 256,
                        ap=[[4096, 128], [1, 2], [512, 2], [1, 255]],
                    )
                    nc.vector.tensor_tensor(proda[:, t, :, :, :], xview, yva[:, t, :, :, :], MULT)
                yva_f = yva.rearrange("p t a b j -> p (t a b j)")
                ysqa = mlp.tile([128, 2, 2, 2, 255], BF16, tag="ysqa", name=f"ysqa_{s}_{ci}")
                ysqa_f = ysqa.rearrange("p t a b j -> p (t a b j)")
                if YSQ_GPS[ci]:
                    nc.gpsimd.tensor_tensor(ysqa_f, yva_f, yva_f, MULT)
                else:
                    nc.vector.tensor_tensor(ysqa_f, yva_f, yva_f, MULT)
                pend = (s, ci, proda, ysqa)

            # prefetch next image's row tiles
            if s + 1 < NSAMP:
                fetch(s + 1)

        # drain: last image's final ci and tail
        ct_d(*pend[:3])
        ct_yx(pend[0], pend[1], pend[3])
        emit_sim(NSAMP - 1)
        emit_fold(NSAMP - 1)

    nc.finalize()
    return nc


def make_weight_inputs(W1, b1, W2, b2, W3, b3):
    """Host-side block-diagonal weight construction."""
    W1 = np.asarray(W1, np.float32)
    W2 = np.asarray(W2, np.float32)
    W3 = np.asarray(W3, np.float32)
    b1 = np.asarray(b1, np.float32)
    b2 = np.asarray(b2, np.float32)
    b3 = np.asarray(b3, np.float32)
    # partition orders: image/z3 rows p = 32k+g ; h1/h2 rows q = 32c+g
    l1w = np.zeros((128, 4, 128), np.float32)
    l2w = np.zeros((128, 128), np.float32)
    l3w = np.zeros((128, 4, 128), np.float32)
    b3v = np.zeros((128, 4), np.float32)
    cwm = np.zeros((128, 4, 128), np.float32)
    for g in range(32):
        for l in range(4):
            for k in range(4):
                for c in range(4):
                    l1w[32 * k + g, l, 32 * c + g] = W1[4 * k + l, c]
                    l3w[32 * c + g, l, 32 * k + g] = W3[c, 4 * k + l]
                b3v[32 * k + g, l] = b3[4 * k + l]
        for k in range(4):
            for ci in range(4):
                cwm[32 * k + g, ci, 4 * g + ci] = 1.0
        for c in range(4):
            for c2 in range(4):
                l2w[32 * c + g, 32 * c2 + g] = W2[c, c2]
    b1v = np.repeat(b1, 32).reshape(128, 1).astype(np.float32)
    b2v = np.repeat(b2, 32).reshape(128, 1).astype(np.float32)
    import ml_dtypes

    bf = ml_dtypes.bfloat16
    return {
        "l1w": l1w.astype(bf), "l2w": l2w.astype(bf), "l3w": l3w.astype(bf),
        "b3v": b3v, "cw4": cwm.astype(bf), "b1v": b1v, "b2v": b2v,
    }


_NC = None


def get_nc():
    global _NC
    if _NC is None:
        _NC = build_nc()
    return _NC


def _bf16():
    import ml_dtypes

    return ml_dtypes.bfloat16


def gather_rows(img_n):
    """(n,512,512) f32 -> (n,128,8,2,256) bf16: [p=32k+g, li, t, j] = img[16g+k+2li, 2j+t]."""
    n = img_n.shape[0]
    pad = np.zeros((n, IMG + 4, IMG), np.float32)
    pad[:, :IMG, :] = img_n
    p = np.arange(128)
    li = np.arange(8)
    rows_idx = 16 * (p[:, None] % 32) + (p[:, None] // 32) + 2 * li[None, :]
    out = pad[:, rows_idx, :]                       # (n,128,8,512)
    out = out.reshape(n, 128, 8, 256, 2).transpose(0, 1, 2, 4, 3)  # (n,128,8,2,256)
    return np.ascontiguousarray(out).astype(_bf16())


def make_in_maps(img, wts):
    in_maps = []
    for c in range(NCORES):
        chunk = img[c * NSAMP : (c + 1) * NSAMP]
        m = {"img4b": gather_rows(chunk)}
        m.update(wts)
        in_maps.append(m)
    return in_maps


def kernel(img, W1, b1, W2, b2, W3, b3):
    from concourse.bass_utils import run_bass_kernel_spmd

    img = np.asarray(img, np.float32).reshape(32, IMG, IMG)
    wts = make_weight_inputs(W1, b1, W2, b2, W3, b3)
    nc = get_nc()
    in_maps = make_in_maps(img, wts)
    res = run_bass_kernel_spmd(nc, in_maps, list(range(NCORES)))
    out = np.concatenate([np.asarray(res.results[i]["out4"]) for i in range(NCORES)], axis=0)
    return out.astype(np.float32)


# revision 14
# speedup vs baseline: 1.0941x; 1.0410x over previous
"""Trainium2 Bass kernel for nn_Classical_autoencoder (patch MLP autoencoder + cosine fold).

Contract: kernel(**inputs) takes FULL inputs (img (32,1,512,512), W1 (16,4), b1 (4,),
W2 (4,4), b2 (4,), W3 (4,16), b3 (16,)) and returns the FULL (32,512,512) f32 output.
Internally: pure data-parallel over 8 NeuronCores, 4 images per core.

Math (per image):
  patches x = im2col(img, 4x4, stride 2)           # (255*255, 16)
  y = relu(relu(relu(x@W1+b1)@W2+b2)@W3+b3)        # (P, 16)
  S[i,j] = x.y / (max(|x|,eps)*max(|y|,eps))       # (255,255)
  out[r,c] = mean of S[i,j] for i in {r//2-1, r//2} & [0,255), j likewise
  (the overlapping fold with k=4,s=2 reduces exactly to this 2-tap box filter
   on S, upsampled 2x with 2x2-constant blocks)

V4 layout / engine plan (vs v3):
  img4c dropped: cols 2j+2+t are just rtb[..., j+1] (shift-by-one view), so
      one 512KB row tile per image instead of two.
  products: one 4-free-dim DVE op per ci (overlapping jo/j strides on the
      rtb read) instead of 4 ops -> 12 fewer DVE dispatches per image.
  cw4[32k+g, ci, 4g+ci] gather: ct matmul outputs land at partition q=4g+ci
      = patch rows (2q, 2q+1), i.e. already in fold row-pair order. The
      whole DRAM-bounce reorg of v3 (5 DMAs/image) disappears; all 4 ci
      accumulate into one full-width PSUM bank per quantity.
  single fused output DMA per image (row duplication via 0-stride read).
"""

import sys

for _p in ("/opt/trn_rl_repo", "/root/.axon_site/_ro/trn_rl_repo"):
    if _p not in sys.path:
        sys.path.append(_p)

from contextlib import ExitStack

import numpy as np

import concourse.bass as bass
import concourse.tile as tile
from concourse import bacc, mybir

F32 = mybir.dt.float32
BF16 = mybir.dt.bfloat16
ADD = mybir.AluOpType.add
MULT = mybir.AluOpType.mult
MAX = mybir.AluOpType.max

IMG = 512
OH = 255
NSAMP = 4
NCORES = 8

# GPSIMD cannot read PSUM (BIR verifier) -> all relus on ACT/DVE.
# ysq is SBUF-only, so GpSimd can take a share of it.
YSQ_GPS = (True, True, True, True)  # per ci


def build_nc() -> bass.Bass:
    nc = bacc.Bacc()

    img4b = nc.declare_dram_parameter("img4b", [NSAMP, 128, 8, 2, 256], BF16, isOutput=False)[:]
    l1w = nc.declare_dram_parameter("l1w", [128, 4, 128], BF16, isOutput=False)[:]
    l2w = nc.declare_dram_parameter("l2w", [128, 128], BF16, isOutput=False)[:]
    l3w = nc.declare_dram_parameter("l3w", [128, 4, 128], BF16, isOutput=False)[:]
    b3v = nc.declare_dram_parameter("b3v", [128, 4], F32, isOutput=False)[:]
    cw4 = nc.declare_dram_parameter("cw4", [128, 4, 128], BF16, isOutput=False)[:]
    b1v = nc.declare_dram_parameter("b1v", [128, 1], F32, isOutput=False)[:]
    b2v = nc.declare_dram_parameter("b2v", [128, 1], F32, isOutput=False)[:]
    out4 = nc.declare_dram_parameter("out4", [NSAMP, IMG, IMG], BF16, isOutput=True)[:]

    with ExitStack() as ctx:
        tc = ctx.enter_context(tile.TileContext(nc))
        consts = ctx.enter_context(tc.tile_pool(name="consts", bufs=1))
        rows = ctx.enter_context(tc.tile_pool(name="rows", bufs=2))
        xsq = ctx.enter_context(tc.tile_pool(name="xsq", bufs=2))
        mlp = ctx.enter_context(tc.tile_pool(name="mlp", bufs=3))
        tailp = ctx.enter_context(tc.tile_pool(name="tailp", bufs=2))
        foldp = ctx.enter_context(tc.tile_pool(name="foldp", bufs=2))
        psz = ctx.enter_context(tc.tile_pool(name="psz", bufs=2, space="PSUM"))
        psz3 = ctx.enter_context(tc.tile_pool(name="psz3", bufs=3, space="PSUM"))
        psct = ctx.enter_context(tc.tile_pool(name="psct", bufs=1, space="PSUM"))

        rt_tiles = {}

        def fetch(s):
            # split li 0-1 first so ci=0 compute can start before the full tile lands
            rtb = rows.tile([128, 8, 2, 256], BF16, tag="rtb", name=f"rtb{s}")
            nc.sync.dma_start(out=rtb[:, 0:2], in_=img4b[s, :, 0:2, :, :])
            nc.sync.dma_start(out=rtb[:, 2:8], in_=img4b[s, :, 2:8, :, :])
            rt_tiles[s] = rtb

        # ---- constants (first fetch launched ahead; l1w/b1 next on the
        # sync queue; the rest spread over idle engine queues) ----
        fetch(0)
        l1w_t = consts.tile([128, 4, 128], BF16)
        nc.sync.dma_start(out=l1w_t, in_=l1w[:, :, :])
        b1_t = consts.tile([128, 1], F32)
        nc.sync.dma_start(out=b1_t, in_=b1v[:, :])
        l2w_t = consts.tile([128, 128], BF16)
        nc.gpsimd.dma_start(out=l2w_t, in_=l2w[:, :])
        b2_t = consts.tile([128, 1], F32)
        nc.gpsimd.dma_start(out=b2_t, in_=b2v[:, :])
        l3w_t = consts.tile([128, 4, 128], BF16)
        nc.scalar.dma_start(out=l3w_t, in_=l3w[:, :, :])
        b3_t = consts.tile([128, 4], F32)
        nc.scalar.dma_start(out=b3_t, in_=b3v[:, :])
        cw_t = consts.tile([128, 4, 128], BF16)
        nc.gpsimd.dma_start(out=cw_t, in_=cw4[:, :, :])
        eps_t = consts.tile([128, 1], F32)
        nc.vector.memset(eps_t, 1e-12)

        def relu(out, z, bias):
            nc.scalar.activation(out, z, mybir.ActivationFunctionType.Relu, bias=bias)

        # per-sample live state for the cross-image pipeline
        ct_banks = {}
        s1_tiles = {}
        sim_tiles = {}

        def ct_d(ps, ci, proda):
            ctd_b = ct_banks[ps][0]
            for l in range(4):
                nc.tensor.matmul(
                    ctd_b[:, 0:510], cw_t[:, ci, :], proda[:, l % 2, l // 2, :, :],
                    start=(ci == 0 and l == 0), stop=(ci == 3 and l == 3),
                )

        def ct_yx(ps, ci, ysqa):
            _, ctx_b, cty_b = ct_banks[ps]
            for l in range(4):
                nc.tensor.matmul(
                    cty_b[:, 0:510], cw_t[:, ci, :], ysqa[:, l % 2, l // 2, :, :],
                    start=(ci == 0 and l == 0), stop=(ci == 3 and l == 3),
                )
            nc.tensor.matmul(
                ctx_b[:, 0:510], cw_t[:, ci, :], s1_tiles[ps][:, 2 * ci : 2 * ci + 2, :],
                start=(ci == 0), stop=(ci == 3),
            )

        def alloc_ct(ps):
            # out partition q = 4g+ci holds patch rows (2q, 2q+1) -> fold order.
            ct_banks[ps] = (
                psct.tile([128, 512], F32, tag="ctd", name=f"ctd{ps}"),
                psct.tile([128, 512], F32, tag="ctx", name=f"ctx{ps}"),
                psct.tile([128, 512], F32, tag="cty", name=f"cty{ps}"),
            )

        def emit_sim(ps):
            # cosine tail (PSUM readers): sim = ctd / (4|x| * |y|).
            # The /4 pre-scales the fold average.
            ctd_b, ctx_b, cty_b = ct_banks.pop(ps)
            s1_tiles.pop(ps)
            sx = tailp.tile([128, 510], F32, tag="sx")
            nc.scalar.activation(
                sx, ctx_b[:, 0:510],
                mybir.ActivationFunctionType.Sqrt, bias=eps_t[:, :], scale=16.0,
            )
            sy = tailp.tile([128, 510], F32, tag="sy")
            nc.scalar.activation(
                sy, cty_b[:, 0:510],
                mybir.ActivationFunctionType.Sqrt, bias=eps_t[:, :],
            )
            m_ = tailp.tile([128, 510], F32, tag="m_")
            nc.vector.tensor_tensor(m_, sx, sy, MULT)
            r_ = tailp.tile([128, 510], F32, tag="r_")
            nc.vector.reciprocal_approx_fast(r_, m_)
            simacc = tailp.tile([128, 2, 255], F32, tag="simacc")
            nc.vector.tensor_tensor(
                simacc.rearrange("p a j -> p (a j)"), ctd_b[:, 0:510], r_, MULT,
            )
            sim_tiles[ps] = simacc

        def emit_fold(ps, drain=False):
            # fold cols: R[q,u,v] = S[2q+u,v-1]+S[2q+u,v], edges doubled
            # drain=True (last image): spread DMAs over idle queues
            q_edge = nc.scalar if drain else nc.sync
            q_rfs = nc.gpsimd if drain else nc.sync
            q_out2 = nc.scalar if drain else nc.sync
            simacc = sim_tiles.pop(ps)
            rf = foldp.tile([128, 2, 256], F32, tag="rf")
            nc.vector.tensor_tensor(
                rf[:, :, 1:255], simacc[:, :, 0:254], simacc[:, :, 1:255], ADD
            )
            nc.scalar.activation(
                rf[:, :, 0:1], simacc[:, :, 0:1],
                mybir.ActivationFunctionType.Copy, scale=2.0,
            )
            nc.scalar.activation(
                rf[:, :, 255:256], simacc[:, :, 254:255],
                mybir.ActivationFunctionType.Copy, scale=2.0,
            )
            # S row 255 doesn't exist -> row 511 folds 2*R[254]
            q_edge.dma_start(out=rf[127:128, 1, :], in_=rf[127:128, 0, :])
            # fold rows: T[2q+u] = R[2q+u-1]+R[2q+u] via partition-shifted
            # copy of odd rows: rfs[q] = R[2q-1] (rfs[0]=R[0])
            rfs = foldp.tile([128, 256], F32, tag="rfs")
            q_rfs.dma_start(out=rfs[1:128, :], in_=rf[0:127, 1, :])
            q_rfs.dma_start(out=rfs[0:1, :], in_=rf[0:1, 0, :])
            tf = foldp.tile([128, 2, 256], BF16, tag="tf")
            nc.vector.tensor_tensor(tf[:, 1, :], rf[:, 0, :], rf[:, 1, :], ADD)
            nc.vector.tensor_tensor(tf[:, 0, :], rfs, rf[:, 0, :], ADD)
            # 2x2 upsample: duplicate cols on-chip, two row-pair output DMAs
            tf2 = foldp.tile([128, 2, 512], BF16, tag="tf2")
            tf2r = tf2.rearrange("p u (v cv) -> p u cv v", cv=2)
            nc.scalar.activation(
                tf2r[:, :, 0, :], tf, mybir.ActivationFunctionType.Copy
            )
            nc.vector.tensor_copy(tf2r[:, :, 1, :], tf)
            for ru in range(2):
                q = q_out2 if ru else nc.sync
                q.dma_start(
                    out=bass.AP(
                        tensor=out4.tensor,
                        offset=out4.offset + ps * IMG * IMG + ru * IMG,
                        ap=[[4 * IMG, 128], [2 * IMG, 2], [1, IMG]],
                    ),
                    in_=bass.AP(
                        tensor=tf2.tensor,
                        offset=tf2.offset,
                        ap=[[1024, 128], [512, 2], [1, 512]],
                    ),
                )

        # ---- PE p-state warmup: junk matmuls under the startup DMA wait ----
        junk = consts.tile([128, 512], BF16)
        nc.vector.memset(junk, 0.0)
        for wi in range(5):
            zw = psz3.tile([128, 512], F32, tag="z3", name=f"warm{wi}")
            nc.tensor.matmul(zw, junk[:, 0:128], junk, start=True, stop=True)

        # Software pipeline: ci's ct matmuls are emitted TWO ci slots later
        # (crossing image boundaries) so the relu->products chain never stalls
        # the PE; the previous image's tail rides along at ci 2-3.
        pendq = []
        for s in range(NSAMP):
            rtb = rt_tiles.pop(s)
            rtb_flat = rtb.rearrange("p a t j -> p (a t j)")

            # ---- |x|^2 box filter prep (all li at once) ----
            sq = xsq.tile([128, 8, 2, 256], BF16, tag="sq")
            nc.vector.tensor_tensor(
                sq.rearrange("p a t j -> p (a t j)"), rtb_flat, rtb_flat, MULT
            )
            c2 = xsq.tile([128, 8, 256], BF16, tag="c2")
            nc.vector.tensor_tensor(c2, sq[:, :, 0, :], sq[:, :, 1, :], ADD)
            s1 = xsq.tile([128, 8, 255], BF16, tag="s1")
            nc.vector.tensor_tensor(s1, c2[:, :, 0:255], c2[:, :, 1:256], ADD)
            s1_tiles[s] = s1

            def xv(ci, l):
                # [128, 2(li), 255(j)] view: patch cols j at image col 2j+l
                return rtb[:, 2 * ci : 2 * ci + 2, l % 2, (l // 2) : (l // 2) + 255]

            for ci in range(4):
                if ci == 2:
                    # previous image's PSUM readers, then reclaim its banks
                    if s > 0:
                        emit_sim(s - 1)
                    alloc_ct(s)
                if ci == 3 and s > 0:
                    emit_fold(s - 1)
                # ---- layer 1 ----
                z1 = psz.tile([128, 510], F32, tag="z", name=f"z1_{s}_{ci}")
                for l in range(4):
                    nc.tensor.matmul(
                        z1, l1w_t[:, l, :], xv(ci, l), start=(l == 0), stop=(l == 3)
                    )
                if len(pendq) >= 2:
                    ct_d(*pendq[0][:3])
                h1 = mlp.tile([128, 510], BF16, tag="h1", name=f"h1_{s}_{ci}")
                relu(h1, z1, b1_t[:, :])
                # ---- layer 2 ----
                z2 = psz.tile([128, 510], F32, tag="z", name=f"z2_{s}_{ci}")
                nc.tensor.matmul(z2, l2w_t[:, :], h1, start=True, stop=True)
                if len(pendq) >= 2:
                    p0 = pendq.pop(0)
                    ct_yx(p0[0], p0[1], p0[3])
                h2 = mlp.tile([128, 510], BF16, tag="h2", name=f"h2_{s}_{ci}")
                relu(h2, z2, b2_t[:, :])
                # ---- layer 3 + products ----
                yva = mlp.tile([128, 2, 2, 2, 255], BF16, tag="yva", name=f"yva_{s}_{ci}")
                for l in range(4):
                    z3 = psz3.tile([128, 510], F32, tag="z3", name=f"z3_{s}_{ci}_{l}")
                    nc.tensor.matmul(z3, l3w_t[:, l, :], h2, start=True, stop=True)
                    relu(
                        yva[:, l % 2, l // 2, :, :].rearrange("p a j -> p (a j)"),
                        z3, b3_t[:, l : l + 1],
                    )
                # products: one op per jo (clean non-overlapping APs)
                proda = mlp.tile([128, 2, 2, 2, 255], BF16, tag="proda", name=f"proda_{s}_{ci}")
                for jo in range(2):
                    xview = bass.AP(
                        tensor=rtb.tensor,
                        offset=rtb.offset + 2 * ci * 512 + jo,
                        ap=[[4096, 128], [256, 2], [512, 2], [1, 255]],
                    )
                    nc.vector.tensor_tensor(
                        proda[:, :, jo, :, :], xview, yva[:, :, jo, :, :], MULT
                    )
                yva_f = yva.rearrange("p t a b j -> p (t a b j)")
                ysqa = mlp.tile([128, 2, 2, 2, 255], BF16, tag="ysqa", name=f"ysqa_{s}_{ci}")
                ysqa_f = ysqa.rearrange("p t a b j -> p (t a b j)")
                if YSQ_GPS[ci]:
                    nc.gpsimd.tensor_tensor(ysqa_f, yva_f, yva_f, MULT)
                else:
                    nc.vector.tensor_tensor(ysqa_f, yva_f, yva_f, MULT)
                pendq.append((s, ci, proda, ysqa))

            # prefetch next image's row tiles
            if s + 1 < NSAMP:
                fetch(s + 1)

        # drain: last two pending ci groups, then the last image's tail
        for p0 in pendq:
            ct_d(*p0[:3])
            ct_yx(p0[0], p0[1], p0[3])
        emit_sim(NSAMP - 1)
        emit_fold(NSAMP - 1, drain=True)

    nc.finalize()
    return nc


def make_weight_inputs(W1, b1, W2, b2, W3, b3):
    """Host-side block-diagonal weight construction."""
    W1 = np.asarray(W1, np.float32)
    W2 = np.asarray(W2, np.float32)
    W3 = np.asarray(W3, np.float32)
    b1 = np.asarray(b1, np.float32)
    b2 = np.asarray(b2, np.float32)
    b3 = np.asarray(b3, np.float32)
    # partition orders: image/z3 rows p = 32k+g ; h1/h2 rows q = 32c+g
    l1w = np.zeros((128, 4, 128), np.float32)
    l2w = np.zeros((128, 128), np.float32)
    l3w = np.zeros((128, 4, 128), np.float32)
    b3v = np.zeros((128, 4), np.float32)
    cwm = np.zeros((128, 4, 128), np.float32)
    for g in range(32):
        for l in range(4):
            for k in range(4):
                for c in range(4):
                    l1w[32 * k + g, l, 32 * c + g] = W1[4 * k + l, c]
                    l3w[32 * c + g, l, 32 * k + g] = W3[c, 4 * k + l]
                b3v[32 * k + g, l] = b3[4 * k + l]
        for k in range(4):
            for ci in range(4):
                cwm[32 * k + g, ci, 4 * g + ci] = 1.0
        for c in range(4):
            for c2 in range(4):
                l2w[32 * c + g, 32 * c2 + g] = W2[c, c2]
    b1v = np.repeat(b1, 32).reshape(128, 1).astype(np.float32)
    b2v = np.repeat(b2, 32).reshape(128, 1).astype(np.float32)
    import ml_dtypes

    bf = ml_dtypes.bfloat16
    return {
        "l1w": l1w.astype(bf), "l2w": l2w.astype(bf), "l3w": l3w.astype(bf),
        "b3v": b3v, "cw4": cwm.astype(bf), "b1v": b1v, "b2v": b2v,
    }


_NC = None


def get_nc():
    global _NC
    if _NC is None:
        _NC = build_nc()
    return _NC


def _bf16():
    import ml_dtypes

    return ml_dtypes.bfloat16


def gather_rows(img_n):
    """(n,512,512) f32 -> (n,128,8,2,256) bf16: [p=32k+g, li, t, j] = img[16g+k+2li, 2j+t]."""
    n = img_n.shape[0]
    pad = np.zeros((n, IMG + 4, IMG), np.float32)
    pad[:, :IMG, :] = img_n
    p = np.arange(128)
    li = np.arange(8)
    rows_idx = 16 * (p[:, None] % 32) + (p[:, None] // 32) + 2 * li[None, :]
    out = pad[:, rows_idx, :]                       # (n,128,8,512)
    out = out.reshape(n, 128, 8, 256, 2).transpose(0, 1, 2, 4, 3)  # (n,128,8,2,256)
    return np.ascontiguousarray(out).astype(_bf16())


def make_in_maps(img, wts):
    in_maps = []
    for c in range(NCORES):
        chunk = img[c * NSAMP : (c + 1) * NSAMP]
        m = {"img4b": gather_rows(chunk)}
        m.update(wts)
        in_maps.append(m)
    return in_maps


def kernel(img, W1, b1, W2, b2, W3, b3):
    from concourse.bass_utils import run_bass_kernel_spmd

    img = np.asarray(img, np.float32).reshape(32, IMG, IMG)
    wts = make_weight_inputs(W1, b1, W2, b2, W3, b3)
    nc = get_nc()
    in_maps = make_in_maps(img, wts)
    res = run_bass_kernel_spmd(nc, in_maps, list(range(NCORES)))
    out = np.concatenate([np.asarray(res.results[i]["out4"]) for i in range(NCORES)], axis=0)
    return out.astype(np.float32)


# revision 20
# speedup vs baseline: 1.2385x; 1.1319x over previous
"""Trainium2 Bass kernel for nn_Classical_autoencoder (patch MLP autoencoder + cosine fold).

Contract: kernel(**inputs) takes FULL inputs (img (32,1,512,512), W1 (16,4), b1 (4,),
W2 (4,4), b2 (4,), W3 (4,16), b3 (16,)) and returns the FULL (32,512,512) f32 output.
Internally: pure data-parallel over 8 NeuronCores, 4 images per core.

Math (per image):
  patches x = im2col(img, 4x4, stride 2)           # (255*255, 16)
  y = relu(relu(relu(x@W1+b1)@W2+b2)@W3+b3)        # (P, 16)
  S[i,j] = x.y / (max(|x|,eps)*max(|y|,eps))       # (255,255)
  out[r,c] = mean of S[i,j] for i in {r//2-1, r//2} & [0,255), j likewise
  (the overlapping fold with k=4,s=2 reduces exactly to this 2-tap box filter
   on S, upsampled 2x with 2x2-constant blocks)

V4 layout / engine plan (vs v3):
  img4c dropped: cols 2j+2+t are just rtb[..., j+1] (shift-by-one view), so
      one 512KB row tile per image instead of two.
  products: one 4-free-dim DVE op per ci (overlapping jo/j strides on the
      rtb read) instead of 4 ops -> 12 fewer DVE dispatches per image.
  cw4[32k+g, ci, 4g+ci] gather: ct matmul outputs land at partition q=4g+ci
      = patch rows (2q, 2q+1), i.e. already in fold row-pair order. The
      whole DRAM-bounce reorg of v3 (5 DMAs/image) disappears; all 4 ci
      accumulate into one full-width PSUM bank per quantity.
  single fused output DMA per image (row duplication via 0-stride read).
"""

import sys

for _p in ("/opt/trn_rl_repo", "/root/.axon_site/_ro/trn_rl_repo"):
    if _p not in sys.path:
        sys.path.append(_p)

from contextlib import ExitStack

import numpy as np

import concourse.bass as bass
import concourse.tile as tile
from concourse import bacc, mybir

F32 = mybir.dt.float32
BF16 = mybir.dt.bfloat16
ADD = mybir.AluOpType.add
MULT = mybir.AluOpType.mult
MAX = mybir.AluOpType.max

IMG = 512
OH = 255
NSAMP = 4
NCORES = 8

# GPSIMD cannot read PSUM (BIR verifier) -> all relus on ACT/DVE.
# ysq is SBUF-only, so GpSimd can take a share of it.
YSQ_GPS = (True, True, True, True)  # per ci


def build_nc() -> bass.Bass:
    nc = bacc.Bacc()

    img4b = nc.declare_dram_parameter("img4b", [NSAMP, 128, 8, 2, 256], BF16, isOutput=False)[:]
    l1w = nc.declare_dram_parameter("l1w", [128, 4, 128], BF16, isOutput=False)[:]
    l2w = nc.declare_dram_parameter("l2w", [128, 128], BF16, isOutput=False)[:]
    l3w = nc.declare_dram_parameter("l3w", [128, 4, 128], BF16, isOutput=False)[:]
    b3v = nc.declare_dram_parameter("b3v", [128, 4], F32, isOutput=False)[:]
    cw4 = nc.declare_dram_parameter("cw4", [128, 4, 128], BF16, isOutput=False)[:]
    fw = nc.declare_dram_parameter("fw", [128, 4, 128], BF16, isOutput=False)[:]
    b1v = nc.declare_dram_parameter("b1v", [128, 1], F32, isOutput=False)[:]
    b2v = nc.declare_dram_parameter("b2v", [128, 1], F32, isOutput=False)[:]
    out4 = nc.declare_dram_parameter("out4", [NSAMP, IMG, IMG], BF16, isOutput=True)[:]

    with ExitStack() as ctx:
        tc = ctx.enter_context(tile.TileContext(nc))
        consts = ctx.enter_context(tc.tile_pool(name="consts", bufs=1))
        rows = ctx.enter_context(tc.tile_pool(name="rows", bufs=2))
        xsq = ctx.enter_context(tc.tile_pool(name="xsq", bufs=2))
        mlp = ctx.enter_context(tc.tile_pool(name="mlp", bufs=3))
        tailp = ctx.enter_context(tc.tile_pool(name="tailp", bufs=2))
        foldp = ctx.enter_context(tc.tile_pool(name="foldp", bufs=2))
        psz = ctx.enter_context(tc.tile_pool(name="psz", bufs=2, space="PSUM"))
        psz3 = ctx.enter_context(tc.tile_pool(name="psz3", bufs=3, space="PSUM"))
        psct = ctx.enter_context(tc.tile_pool(name="psct", bufs=1, space="PSUM"))

        rt_tiles = {}

        def fetch(s):
            # split li 0-1 first so ci=0 compute can start before the full tile lands
            rtb = rows.tile([128, 8, 2, 256], BF16, tag="rtb", name=f"rtb{s}")
            nc.sync.dma_start(out=rtb[:, 0:2], in_=img4b[s, :, 0:2, :, :])
            nc.sync.dma_start(out=rtb[:, 2:8], in_=img4b[s, :, 2:8, :, :])
            rt_tiles[s] = rtb

        # ---- constants (first fetch launched ahead; l1w/b1 next on the
        # sync queue; the rest spread over idle engine queues) ----
        fetch(0)
        l1w_t = consts.tile([128, 4, 128], BF16)
        nc.sync.dma_start(out=l1w_t, in_=l1w[:, :, :])
        b1_t = consts.tile([128, 1], F32)
        nc.sync.dma_start(out=b1_t, in_=b1v[:, :])
        l2w_t = consts.tile([128, 128], BF16)
        nc.gpsimd.dma_start(out=l2w_t, in_=l2w[:, :])
        b2_t = consts.tile([128, 1], F32)
        nc.gpsimd.dma_start(out=b2_t, in_=b2v[:, :])
        l3w_t = consts.tile([128, 4, 128], BF16)
        nc.scalar.dma_start(out=l3w_t, in_=l3w[:, :, :])
        b3_t = consts.tile([128, 4], F32)
        nc.scalar.dma_start(out=b3_t, in_=b3v[:, :])
        cw_t = consts.tile([128, 4, 128], BF16)
        nc.gpsimd.dma_start(out=cw_t, in_=cw4[:, :, :])
        fw_t = consts.tile([128, 4, 128], BF16)
        nc.gpsimd.dma_start(out=fw_t, in_=fw[:, :, :])
        eps_t = consts.tile([128, 1], F32)
        nc.vector.memset(eps_t, 1e-12)

        def relu(out, z, bias):
            nc.scalar.activation(out, z, mybir.ActivationFunctionType.Relu, bias=bias)

        # per-sample live state for the cross-image pipeline
        ct_banks = {}
        s1_tiles = {}
        sim_tiles = {}

        def ct_d(ps, ci, proda):
            ctd_b = ct_banks[ps][0]
            for l in range(4):
                nc.tensor.matmul(
                    ctd_b[:, 0:510], cw_t[:, ci, :], proda[:, l % 2, l // 2, :, :],
                    start=(ci == 0 and l == 0), stop=(ci == 3 and l == 3),
                )

        def ct_yx(ps, ci, ysqa):
            _, ctx_b, cty_b = ct_banks[ps]
            for l in range(4):
                nc.tensor.matmul(
                    cty_b[:, 0:510], cw_t[:, ci, :], ysqa[:, l % 2, l // 2, :, :],
                    start=(ci == 0 and l == 0), stop=(ci == 3 and l == 3),
                )
            nc.tensor.matmul(
                ctx_b[:, 0:510], cw_t[:, ci, :], s1_tiles[ps][:, 2 * ci : 2 * ci + 2, :],
                start=(ci == 0), stop=(ci == 3),
            )

        def alloc_ct(ps):
            # out partition q = 4g+ci holds patch rows (2q, 2q+1) -> fold order.
            ct_banks[ps] = (
                psct.tile([128, 512], F32, tag="ctd", name=f"ctd{ps}"),
                psct.tile([128, 512], F32, tag="ctx", name=f"ctx{ps}"),
                psct.tile([128, 512], F32, tag="cty", name=f"cty{ps}"),
            )

        def emit_sim(ps):
            # cosine tail, PSUM readers only (3 ACT ops): frees all three ct
            # banks without waiting on any DVE chain.
            ctd_b, ctx_b, cty_b = ct_banks.pop(ps)
            s1_tiles.pop(ps)
            sx = tailp.tile([128, 510], F32, tag="sx")
            nc.scalar.activation(
                sx, ctx_b[:, 0:510],
                mybir.ActivationFunctionType.Sqrt, bias=eps_t[:, :], scale=16.0,
            )
            sy = tailp.tile([128, 510], F32, tag="sy")
            nc.scalar.activation(
                sy, cty_b[:, 0:510],
                mybir.ActivationFunctionType.Sqrt, bias=eps_t[:, :],
            )
            d_ = tailp.tile([128, 510], F32, tag="d_")
            nc.scalar.activation(d_, ctd_b[:, 0:510], mybir.ActivationFunctionType.Copy)
            sim_tiles[ps] = (sx, sy, d_)

        def emit_fold(ps, drain=False):
            # sim = ctd / (4|x| * |y|); the /4 pre-scales the fold average.
            q_out2 = nc.scalar if drain else nc.sync
            sx, sy, d_ = sim_tiles.pop(ps)
            m_ = tailp.tile([128, 510], F32, tag="m_")
            nc.vector.tensor_tensor(m_, sx, sy, MULT)
            r_ = tailp.tile([128, 510], F32, tag="r_")
            nc.vector.reciprocal_approx_fast(r_, m_)
            simacc = tailp.tile([128, 2, 255], F32, tag="simacc")
            nc.vector.tensor_tensor(
                simacc.rearrange("p a j -> p (a j)"), d_, r_, MULT,
            )
            # fold cols: R[q,u,v] = S[2q+u,v-1]+S[2q+u,v], edges doubled
            rf = foldp.tile([128, 2, 256], BF16, tag="rf")
            nc.vector.tensor_tensor(
                rf[:, :, 1:255], simacc[:, :, 0:254], simacc[:, :, 1:255], ADD
            )
            nc.scalar.activation(
                rf[:, :, 0:1], simacc[:, :, 0:1],
                mybir.ActivationFunctionType.Copy, scale=2.0,
            )
            nc.scalar.activation(
                rf[:, :, 255:256], simacc[:, :, 254:255],
                mybir.ActivationFunctionType.Copy, scale=2.0,
            )
            # fold rows on the PE (no partition-shift DMA):
            #   tfp[q,0,:] = R[2q-1]+R[2q]   (fw0 = superdiag shift, fw1 = I
            #                                 with [0,0]=2 for the R[-1]:=R[0] edge)
            #   tfp[q,1,:] = R[2q]+R[2q+1]   (fw2 = I with [127,127]=2, fw3 = I
            #                                 with [127,127]=0: junk R[255] masked,
            #                                 row 511 folds 2*R[254])
            tfp = psz3.tile([128, 2, 256], F32, tag="z3", name=f"tfp{ps}")
            nc.tensor.matmul(tfp[:, 0, :], fw_t[:, 0, :], rf[:, 1, :], start=True, stop=False)
            nc.tensor.matmul(tfp[:, 0, :], fw_t[:, 1, :], rf[:, 0, :], start=False, stop=True)
            nc.tensor.matmul(tfp[:, 1, :], fw_t[:, 2, :], rf[:, 0, :], start=True, stop=False)
            nc.tensor.matmul(tfp[:, 1, :], fw_t[:, 3, :], rf[:, 1, :], start=False, stop=True)
            # 2x2 upsample straight out of PSUM: duplicate cols on-chip
            tf2 = foldp.tile([128, 2, 512], BF16, tag="tf2")
            tf2r = tf2.rearrange("p u (v cv) -> p u cv v", cv=2)
            nc.scalar.activation(
                tf2r[:, :, 0, :], tfp, mybir.ActivationFunctionType.Copy
            )
            nc.vector.tensor_copy(tf2r[:, :, 1, :], tfp)
            for ru in range(2):
                q = q_out2 if ru else nc.sync
                q.dma_start(
                    out=bass.AP(
                        tensor=out4.tensor,
                        offset=out4.offset + ps * IMG * IMG + ru * IMG,
                        ap=[[4 * IMG, 128], [2 * IMG, 2], [1, IMG]],
                    ),
                    in_=bass.AP(
                        tensor=tf2.tensor,
                        offset=tf2.offset,
                        ap=[[1024, 128], [512, 2], [1, 512]],
                    ),
                )

        # ---- PE p-state warmup: junk matmuls under the startup DMA wait ----
        junk = consts.tile([128, 512], BF16)
        nc.vector.memset(junk, 0.0)
        for wi in range(5):
            zw = psz3.tile([128, 512], F32, tag="z3", name=f"warm{wi}")
            nc.tensor.matmul(zw, junk[:, 0:128], junk, start=True, stop=True)

        # Software pipeline: ci's ct matmuls are emitted TWO ci slots later
        # (crossing image boundaries) so the relu->products chain never stalls
        # the PE; the previous image's tail rides along at ci 2-3.
        pendq = []
        for s in range(NSAMP):
            rtb = rt_tiles.pop(s)

            def xv(ci, l):
                # [128, 2(li), 255(j)] view: patch cols j at image col 2j+l
                return rtb[:, 2 * ci : 2 * ci + 2, l % 2, (l // 2) : (l // 2) + 255]

            for ci in range(4):
                # ---- layer 1 ----
                z1 = psz.tile([128, 510], F32, tag="z", name=f"z1_{s}_{ci}")
                for l in range(4):
                    nc.tensor.matmul(
                        z1, l1w_t[:, l, :], xv(ci, l), start=(l == 0), stop=(l == 3)
                    )
                h1 = mlp.tile([128, 510], BF16, tag="h1", name=f"h1_{s}_{ci}")
                relu(h1, z1, b1_t[:, :])
                # ---- layer 2 ----
                z2 = psz.tile([128, 510], F32, tag="z", name=f"z2_{s}_{ci}")
                nc.tensor.matmul(z2, l2w_t[:, :], h1, start=True, stop=True)
                h2 = mlp.tile([128, 510], BF16, tag="h2", name=f"h2_{s}_{ci}")
                relu(h2, z2, b2_t[:, :])
                # ---- layer 3 + products ----
                yva = mlp.tile([128, 2, 2, 2, 255], BF16, tag="yva", name=f"yva_{s}_{ci}")
                for l in range(4):
                    z3 = psz3.tile([128, 510], F32, tag="z3", name=f"z3_{s}_{ci}_{l}")
                    nc.tensor.matmul(z3, l3w_t[:, l, :], h2, start=True, stop=True)
                    relu(
                        yva[:, l % 2, l // 2, :, :].rearrange("p a j -> p (a j)"),
                        z3, b3_t[:, l : l + 1],
                    )
                # products: one op per jo (clean non-overlapping APs)
                proda = mlp.tile([128, 2, 2, 2, 255], BF16, tag="proda", name=f"proda_{s}_{ci}")
                for jo in range(2):
                    xview = bass.AP(
                        tensor=rtb.tensor,
                        offset=rtb.offset + 2 * ci * 512 + jo,
                        ap=[[4096, 128], [256, 2], [512, 2], [1, 255]],
                    )
                    nc.vector.tensor_tensor(
                        proda[:, :, jo, :, :], xview, yva[:, :, jo, :, :], MULT
                    )
                yva_f = yva.rearrange("p t a b j -> p (t a b j)")
                ysqa = mlp.tile([128, 2, 2, 2, 255], BF16, tag="ysqa", name=f"ysqa_{s}_{ci}")
                ysqa_f = ysqa.rearrange("p t a b j -> p (t a b j)")
                if YSQ_GPS[ci]:
                    nc.gpsimd.tensor_tensor(ysqa_f, yva_f, yva_f, MULT)
                else:
                    nc.vector.tensor_tensor(ysqa_f, yva_f, yva_f, MULT)
                pendq.append((s, ci, proda, ysqa))

                # ---- |x|^2 box filter prep (at ci 1, once rtb is complete) ----
                if ci == 1:
                    rtb_flat = rtb.rearrange("p a t j -> p (a t j)")
                    sq = xsq.tile([128, 8, 2, 256], BF16, tag="sq")
                    nc.vector.tensor_tensor(
                        sq.rearrange("p a t j -> p (a t j)"), rtb_flat, rtb_flat, MULT
                    )
                    c2 = xsq.tile([128, 8, 256], BF16, tag="c2")
                    nc.vector.tensor_tensor(c2, sq[:, :, 0, :], sq[:, :, 1, :], ADD)
                    s1 = xsq.tile([128, 8, 255], BF16, tag="s1")
                    nc.vector.tensor_tensor(s1, c2[:, :, 0:255], c2[:, :, 1:256], ADD)
                    s1_tiles[s] = s1

                # ---- deferred ct groups + previous image's tail ----
                if ci == 2:
                    if s > 0:
                        emit_sim(s - 1)
                    alloc_ct(s)
                if len(pendq) > 2:
                    p0 = pendq.pop(0)
                    ct_d(*p0[:3])
                    ct_yx(p0[0], p0[1], p0[3])
                if ci == 3 and s > 0:
                    emit_fold(s - 1)

            # prefetch next image's row tiles
            if s + 1 < NSAMP:
                fetch(s + 1)

        # drain: last two pending ci groups, then the last image's tail
        for p0 in pendq:
            ct_d(*p0[:3])
            ct_yx(p0[0], p0[1], p0[3])
        emit_sim(NSAMP - 1)
        emit_fold(NSAMP - 1, drain=True)

    nc.finalize()
    return nc


def make_weight_inputs(W1, b1, W2, b2, W3, b3):
    """Host-side block-diagonal weight construction."""
    W1 = np.asarray(W1, np.float32)
    W2 = np.asarray(W2, np.float32)
    W3 = np.asarray(W3, np.float32)
    b1 = np.asarray(b1, np.float32)
    b2 = np.asarray(b2, np.float32)
    b3 = np.asarray(b3, np.float32)
    # partition orders: image/z3 rows p = 32k+g ; h1/h2 rows q = 32c+g
    l1w = np.zeros((128, 4, 128), np.float32)
    l2w = np.zeros((128, 128), np.float32)
    l3w = np.zeros((128, 4, 128), np.float32)
    b3v = np.zeros((128, 4), np.float32)
    cwm = np.zeros((128, 4, 128), np.float32)
    for g in range(32):
        for l in range(4):
            for k in range(4):
                for c in range(4):
                    l1w[32 * k + g, l, 32 * c + g] = W1[4 * k + l, c]
                    l3w[32 * c + g, l, 32 * k + g] = W3[c, 4 * k + l]
                b3v[32 * k + g, l] = b3[4 * k + l]
        for k in range(4):
            for ci in range(4):
                cwm[32 * k + g, ci, 4 * g + ci] = 1.0
        for c in range(4):
            for c2 in range(4):
                l2w[32 * c + g, 32 * c2 + g] = W2[c, c2]
    b1v = np.repeat(b1, 32).reshape(128, 1).astype(np.float32)
    b2v = np.repeat(b2, 32).reshape(128, 1).astype(np.float32)
    # row-fold matmul weights: tfp[q,0]=R[2q-1]+R[2q], tfp[q,1]=R[2q]+R[2q+1]
    fw = np.zeros((128, 4, 128), np.float32)
    for q in range(1, 128):
        fw[q - 1, 0, q] = 1.0          # superdiag shift of rf[:,1]
    eye = np.arange(128)
    fw[eye, 1, eye] = 1.0
    fw[0, 1, 0] = 2.0                  # R[-1] := R[0]
    fw[eye, 2, eye] = 1.0
    fw[127, 2, 127] = 2.0              # row 511 folds 2*R[254]
    fw[eye, 3, eye] = 1.0
    fw[127, 3, 127] = 0.0              # mask junk R[255]
    import ml_dtypes

    bf = ml_dtypes.bfloat16
    return {
        "l1w": l1w.astype(bf), "l2w": l2w.astype(bf), "l3w": l3w.astype(bf),
        "b3v": b3v, "cw4": cwm.astype(bf), "fw": fw.astype(bf),
        "b1v": b1v, "b2v": b2v,
    }


_NC = None


def get_nc():
    global _NC
    if _NC is None:
        _NC = build_nc()
    return _NC


def _bf16():
    import ml_dtypes

    return ml_dtypes.bfloat16


def gather_rows(img_n):
    """(n,512,512) f32 -> (n,128,8,2,256) bf16: [p=32k+g, li, t, j] = img[16g+k+2li, 2j+t]."""
    n = img_n.shape[0]
    pad = np.zeros((n, IMG + 4, IMG), np.float32)
    pad[:, :IMG, :] = img_n
    p = np.arange(128)
    li = np.arange(8)
    rows_idx = 16 * (p[:, None] % 32) + (p[:, None] // 32) + 2 * li[None, :]
    out = pad[:, rows_idx, :]                       # (n,128,8,512)
    out = out.reshape(n, 128, 8, 256, 2).transpose(0, 1, 2, 4, 3)  # (n,128,8,2,256)
    return np.ascontiguousarray(out).astype(_bf16())


def make_in_maps(img, wts):
    in_maps = []
    for c in range(NCORES):
        chunk = img[c * NSAMP : (c + 1) * NSAMP]
        m = {"img4b": gather_rows(chunk)}
        m.update(wts)
        in_maps.append(m)
    return in_maps


def kernel(img, W1, b1, W2, b2, W3, b3):
    from concourse.bass_utils import run_bass_kernel_spmd

    img = np.asarray(img, np.float32).reshape(32, IMG, IMG)
    wts = make_weight_inputs(W1, b1, W2, b2, W3, b3)
    nc = get_nc()
    in_maps = make_in_maps(img, wts)
    res = run_bass_kernel_spmd(nc, in_maps, list(range(NCORES)))
    out = np.concatenate([np.asarray(res.results[i]["out4"]) for i in range(NCORES)], axis=0)
    return out.astype(np.float32)
